# revision 3
# baseline (speedup 1.0000x reference)
"""Trainium2 Bass kernel for nn_BigramHash (hashed-bigram embedding + projection).

Computation (per reference):
    bigram_idx = pad_left0((idx[:, :-1] * 10007 + idx[:, 1:]) % 8192)   # [B, S]
    h = table[bigram_idx]                                               # fp16 [B, S, 48]
    out = h.astype(f32) @ proj_w.T                                      # f32 [B, S, 512]

Strategy (8-core data parallel over batch, 4 rows = 16384 tokens per core):
  - hash on DVE in int32 using (a & 8191) * 1815 + b (mod-2^13-equivalent,
    products < 2^24 so exact on any ALU path), then & 8191, cast to int16.
  - dma_gather(transpose=True) from the 256B-padded table in DRAM lands
    h^T in SBUF: partitions = d_bigram, free dim = tokens.
  - PE: per 128-token tile, lhsT = h^T slice [48, 128] (stationary),
    rhs = proj^T fp16 hi/lo pair [48, 512] accumulated in one PSUM group
    -> out tile [128 tokens, 512] f32 with ~f32 accuracy.
  - PSUM -> SBUF copies split DVE/ACT, then 1 MiB contiguous DMAs to DRAM.
"""

import sys

sys.path.insert(0, "/opt/trn_rl_repo")

import numpy as np

N_CORES = 8
B, S = 32, 4096
BUCKETS, D_BIGRAM, DIM = 8192, 48, 512
ROWS_PER_CORE = B // N_CORES          # 4
NTOK = ROWS_PER_CORE * S              # 16384 tokens per core
ELEM = 128                            # padded table row: 128 fp16 = 256 B
P = 128

CHUNK = 2048                          # tokens per dma_gather
GRP = 4                               # 128-token tiles per output DMA group

_CACHE: dict = {}


def _build(ntok: int, s_row: int, chunk: int, debug: bool = False):
    """Build the per-core Bass module. ntok tokens, rows of s_row tokens."""
    import concourse.mybir as mybir
    import concourse.tile as tile
    from concourse import bacc

    assert ntok % chunk == 0 and chunk % 128 == 0 and s_row % 16 == 0
    nrows = ntok // s_row
    cols = ntok // 16                 # wrapped idx columns
    ccols = chunk // 16               # wrapped idx columns per chunk
    nch = ntok // chunk               # gather chunks
    tpc = chunk // 128                # token-tiles per chunk
    ngrp = ntok // (GRP * 128)        # output DMA groups
    bcol = s_row // 16                # wrapped col of each row-start token

    f16, f32 = mybir.dt.float16, mybir.dt.float32
    i16, i32 = mybir.dt.int16, mybir.dt.int32
    Alu = mybir.AluOpType

    nc = bacc.Bacc("TRN2", target_bir_lowering=False, debug=debug)
    with tile.TileContext(nc) as tc:
        with (
            tc.tile_pool(name="dram", bufs=1, space="DRAM") as dram,
            tc.tile_pool(name="const", bufs=1) as const,
            tc.tile_pool(name="htp", bufs=3) as htp,
            tc.tile_pool(name="psum", bufs=8, space="PSUM") as psp,
            tc.tile_pool(name="outp", bufs=3) as outp,
        ):
            # idx_a/idx_b are host-prewrapped int32: [16, cols] with
            # a[p, c] = ext[c*16 + p], b[p, c] = ext[c*16 + p + 1] where
            # ext = [0] + idx_flat. Pure layout transform on the host.
            idx_a = dram.tile([16, cols], i32, kind="ExternalInput", name="idx_a", uniquify=False)
            idx_b = dram.tile([16, cols], i32, kind="ExternalInput", name="idx_b", uniquify=False)
            table = dram.tile([BUCKETS, ELEM], f16, kind="ExternalInput", name="table_pad", uniquify=False)
            proj_hi = dram.tile([P, DIM], f16, kind="ExternalInput", name="proj_hi", uniquify=False)
            proj_lo = dram.tile([P, DIM], f16, kind="ExternalInput", name="proj_lo", uniquify=False)
            out = dram.tile([ntok, DIM], f32, kind="ExternalOutput", name="out", uniquify=False)

            pj_hi = const.tile([P, DIM], f16)
            pj_lo = const.tile([P, DIM], f16)
            nc.sync.dma_start(pj_hi[:, :], proj_hi[:, :])
            nc.sync.dma_start(pj_lo[:, :], proj_lo[:, :])

            # bigram = ((a & 8191) * 1815 + b) & 8191  (== (a*10007+b) % 8192)
            # Computed on partitions 0-15 (DVE partition base must be 0/32/..),
            # then replicated to all 128 partitions via SBUF->SBUF DMA
            # (gpsimd cores each read their own 16-row replica).
            ia = const.tile([16, cols], i32)
            ib = const.tile([16, cols], i32)
            nc.sync.dma_start(ia[:, :], idx_a[:, :])
            nc.sync.dma_start(ib[:, :], idx_b[:, :])
            tmp = const.tile([16, cols], i32)
            w16 = const.tile([P, cols], i16)
            nc.vector.tensor_scalar(tmp[:, :], ia[:, :], 8191, None, op0=Alu.bitwise_and)
            nc.vector.tensor_scalar(tmp[:, :], tmp[:, :], 1815, None, op0=Alu.mult)
            nc.vector.tensor_tensor(tmp[:, :], tmp[:, :], ib[:, :], op=Alu.add)
            nc.vector.tensor_scalar(tmp[:, :], tmp[:, :], 8191, None, op0=Alu.bitwise_and)
            # int32 -> int16: little-endian low half, stride-2 copy
            tmp16 = tmp.bitcast(i16).rearrange("p (c two) -> p c two", two=2)
            nc.vector.tensor_copy(w16[0:16, :], tmp16[:, :, 0])
            # Row-start tokens use bigram index 0 (left pad).
            w16v = w16.rearrange("p (r c) -> p r c", c=bcol)
            nc.vector.memset(w16v[0:1, :, 0], 0)
            for r in range(1, 8):
                nc.sync.dma_start(w16[16 * r:16 * r + 16, :], w16[0:16, :])

            out_view = out.rearrange("(G j p) o -> G p j o", p=P, j=GRP)
            for c in range(nch):
                ht = htp.tile([P, 1, chunk], f16, name="ht", tag="ht")
                nc.gpsimd.dma_gather(
                    ht[:, :, :],
                    table[:, :],
                    w16[:, c * ccols:(c + 1) * ccols],
                    chunk,
                    chunk,
                    ELEM,
                    transpose=True,
                    single_packet=False,
                )
                for g in range(tpc // GRP):
                    ot = outp.tile([P, GRP, DIM], f32, name="ot", tag="ot")
                    for j in range(GRP):
                        t = g * GRP + j           # token-tile within chunk
                        ti = c * tpc + t          # global token-tile index
                        ps = psp.tile([P, DIM], f32, name="ps", tag="ps")
                        lhsT = ht[0:D_BIGRAM, 0, t * 128:(t + 1) * 128]
                        nc.tensor.matmul(ps[:, :], lhsT, pj_hi[0:D_BIGRAM, :], start=True, stop=False)
                        nc.tensor.matmul(ps[:, :], lhsT, pj_lo[0:D_BIGRAM, :], start=False, stop=True)
                        if ti % 3 == 2:
                            nc.scalar.copy(ot[:, j, :], ps[:, :])
                        else:
                            nc.vector.tensor_copy(ot[:, j, :], ps[:, :])
                    nc.sync.dma_start(out_view[c * (tpc // GRP) + g], ot[:, :, :])
    nc.compile()
    return nc


def _get_nc():
    key = (NTOK, S, CHUNK)
    if key not in _CACHE:
        _CACHE[key] = _build(NTOK, S, CHUNK)
    return _CACHE[key]


def _host_inputs(idx: np.ndarray, table: np.ndarray, proj_w: np.ndarray):
    """Build the per-core input maps (host-side shard + layout glue)."""
    idx = np.asarray(idx)
    table = np.asarray(table, dtype=np.float16)
    proj = np.asarray(proj_w, dtype=np.float32)

    table_pad = np.zeros((BUCKETS, ELEM), np.float16)
    table_pad[:, :D_BIGRAM] = table

    projT = proj.T.astype(np.float32)                    # [48, 512]
    hi = np.zeros((P, DIM), np.float16)
    lo = np.zeros((P, DIM), np.float16)
    hi[:D_BIGRAM] = projT.astype(np.float16)
    lo[:D_BIGRAM] = (projT - hi[:D_BIGRAM].astype(np.float32)).astype(np.float16)

    in_maps = []
    for c in range(N_CORES):
        shard = np.ascontiguousarray(idx[c * ROWS_PER_CORE:(c + 1) * ROWS_PER_CORE]).reshape(-1).astype(np.int32)
        ext = np.empty(NTOK + 1, np.int32)
        ext[0] = 0
        ext[1:] = shard
        # wrapped layout: [16, cols], element (p, c) = ext[c*16 + p]
        idx_a = np.ascontiguousarray(ext[0:NTOK].reshape(-1, 16).T)
        idx_b = np.ascontiguousarray(ext[1:NTOK + 1].reshape(-1, 16).T)
        in_maps.append({
            "idx_a": idx_a,
            "idx_b": idx_b,
            "table_pad": table_pad,
            "proj_hi": hi,
            "proj_lo": lo,
        })
    return in_maps


def kernel(idx, table, proj_w, _trace=False, _trace_kwargs=None):
    from concourse.bass_utils import run_bass_kernel_spmd

    nc = _get_nc()
    in_maps = _host_inputs(idx, table, proj_w)
    res = run_bass_kernel_spmd(
        nc,
        in_maps,
        core_ids=list(range(N_CORES)),
        trace=_trace,
        **(_trace_kwargs or {}),
    )
    outs = [r["out"].reshape(ROWS_PER_CORE, S, DIM) for r in res.results]
    full = np.concatenate(outs, axis=0)
    if _trace:
        return full, res
    return full
